# revision 1
# baseline (speedup 1.0000x reference)
"""Bidirectional LSTM (B=32, T=2048, I=256, H=128/dir) for 8 Trainium2 cores.

Sharding: data-parallel over (batch, direction) - cores 0-3 run the forward
LSTM over 8 batch rows each, cores 4-7 run the backward LSTM over the
host-flipped sequences.

Per core the nonlinear recurrence is solved with block fixed-point
iteration: time is processed in blocks of L=128 steps; within a block,
K_SWEEPS sweeps each recompute all gates with one batched matmul feedback,
apply sigmoid over the whole block at once, run the c-recurrence with the
hardware tensor_tensor_scan, and recompute h.  Error contracts ~3.7x per
sweep; K=3 with the sweep-0 clamp-tanh measures rel-l2 6.6e-3 (absmax
1.9e-2) vs the fp32 reference, under the 2e-2 gate with 3x margin.  K=4
would be rel-l2 ~1e-3 at ~1.4x the time (flip K_SWEEPS if more margin is
ever needed).

Engine-level design (ScalarE is the bottleneck: ~5C activation elements
per sweep per stream; ~93% busy in the cost model):
 - S=4 streams of BS=2 sequences (C=256 gate columns each) whose gate
   accumulators split the 8 PSUM banks; streams are emitted in rounds
   with a one-item stagger so every engine queue always holds ready work
   from other streams between dependent ops of one stream, and xg bursts
   of different streams spread out (keeps the PE p-state ramped).
 - Feedback matmuls in delta form: gates += W_hh @ (h_new - h_old), 4
   matmuls per sweep instead of +/- pairs.  Sweep 0's "delta" is the h
   tile itself ([carry | h_0] vs the implicit all-zero initial guess), so
   there are no guess matmuls and no per-block hs memsets; delta tiles
   keep column 0 == 0 (the carry never changes within a block).
 - x and W_ih in fp16 (halves input DMA; fp16 matmul is full PE rate);
   h/delta feedback in fp16; PSUM accumulates fp32; last sweep runs fp32.
 - z fused to one DVE op: z/2 = (sigmoid(2g) - 0.5) * sigmoid(i); the
   c-scan is linear in z so it just produces c/2, undone for free by the
   tanh's input scale=2.  The g rows of W_ih/W_hh/bias are pre-scaled by
   2 on the host so one batched sigmoid covers all four gate chunks.
 - Whole x preloaded to SBUF in 4 chunked DMAs (contiguous >=1KB runs).
 - Sweep 0's feedback tanh runs as 2*clamp(c/2, +-0.5) on the DVE (fused
   with the h multiply via scalar_tensor_tensor): its error contracts
   ~rho^2 before the output, and it removes 64 tanh instructions from
   ScalarE, the bottleneck engine (cost model: 297us -> 271us).

_build_nc(reps=R) emits R back-to-back repetitions of the kernel (with
per-rep carry resets, so the output stays exact) - used by test.py to
amortize the ~1 ms per-dispatch axon overhead when timing; the graded
kernel() path uses reps=1.
"""

import numpy as np

import concourse.bass as bass
import concourse.bacc as bacc
import concourse.tile as tile
from concourse import mybir
from concourse.bass_utils import run_bass_kernel_spmd

# Problem shapes (hardcoded per contract)
B, T, I, HS = 32, 2048, 256, 256
H = 128          # per-direction hidden
G4 = 4 * H       # 512 stacked gates
NCORES = 8
U = 8            # sequences per core
S = 4            # independent streams per core (pipelining)
BS = U // S      # sequences per stream (4)
L = 128          # time-block length
NBLK = T // L
K_SWEEPS = 3
C = BS * L       # gate columns per stream-block (512)
NHALF = 1        # column groups per stream for intra-stream pipelining
STAGGER = 1      # per-stream item offset
SPLIT_SIG = False  # sigmoid in 2 chunk-group instructions

# gate chunk order inside the 4*H dim: (i, f, o, g); reference order is (i, f, g, o)
PERM = [0, 1, 3, 2]

F32 = mybir.dt.float32
F16 = mybir.dt.float16

_NC_CACHE = {}


def _build_nc(k_sweeps=K_SWEEPS, reps=1):
    nc = bacc.Bacc()
    xt_h = nc.dram_tensor("xt", [2, 128, U * T], F16, kind="ExternalInput")
    wih_h = nc.dram_tensor("wih", [2, 128, G4], F16, kind="ExternalInput")
    whh_h = nc.dram_tensor("whh", [128, G4], F16, kind="ExternalInput")
    bias_h = nc.dram_tensor("bias", [1, G4], F16, kind="ExternalInput")
    out_h = nc.dram_tensor("out", [128, U * T], F32, kind="ExternalOutput")

    sig = mybir.ActivationFunctionType.Sigmoid
    tanh = mybir.ActivationFunctionType.Tanh
    mult = mybir.AluOpType.mult
    add = mybir.AluOpType.add
    sub = mybir.AluOpType.subtract

    with tile.TileContext(nc) as tc:
        with (
            tc.tile_pool(name="singles", bufs=1) as singles,
            tc.tile_pool(name="work", bufs=2) as work,
            tc.tile_pool(name="psum", bufs=1, space="PSUM") as psump,
        ):
            # --- weights / constants ---
            wih_sb = singles.tile([128, 2, G4], F16, tag="wih")
            nc.sync.dma_start(out=wih_sb, in_=wih_h[:, :, :].transpose([1, 0, 2]))
            whh_sb = singles.tile([128, G4], F16, tag="whh")
            nc.sync.dma_start(out=whh_sb, in_=whh_h[:, :])
            bias_sb = singles.tile([1, G4], F16, tag="bias")
            nc.sync.dma_start(out=bias_sb, in_=bias_h[:, :])
            ones_sb = singles.tile([1, C], F16, tag="ones")
            nc.vector.memset(ones_sb.bitcast(mybir.dt.uint16), 0x3C00)

            # gate accumulators: S streams x (8/S) banks = all 8 PSUM banks
            ps = [psump.tile([128, 4, C], F32, tag=f"ps{s}", name=f"ps{s}")
                  for s in range(S)]

            # Warm-up matmuls: consume every lhsT weight tile once so later
            # matmuls inherit the weight-DMA dependencies via PE program
            # order instead of carrying their own sync waits (the LDW
            # instruction has very few wait slots).
            nc.tensor.matmul(ps[0][:, 0, :], lhsT=whh_sb[:, 0:128],
                             rhs=whh_sb[:, 0:C], start=True, stop=True,
                             skip_group_check=True)
            nc.tensor.matmul(ps[0][:, 0, :], lhsT=wih_sb[:, 0, 0:128],
                             rhs=wih_sb[:, 1, 0:C], start=True, stop=True,
                             skip_group_check=True)
            nc.tensor.matmul(ps[0][:, 0, :], lhsT=bias_sb[:, 0:128],
                             rhs=ones_sb, start=True, stop=True,
                             skip_group_check=True)

            # --- x preload: [128, 2(k), BS, T] per stream, 4 t-chunks ---
            xt_r = xt_h[:, :, :].transpose([1, 0, 2]).rearrange(
                "p k (u t) -> p k u t", u=U)
            xts = [singles.tile([128, 2, BS, T], F16, tag=f"xt{s}",
                                name=f"xt{s}") for s in range(S)]
            NCHUNK = 4
            TCH = T // NCHUNK
            for ch in range(NCHUNK):
                for s in range(S):
                    u0 = s * BS
                    for k in range(2):
                        nc.sync.dma_start(
                            out=xts[s][:, k, :, ch * TCH:(ch + 1) * TCH],
                            in_=xt_r[:, k, u0:u0 + BS, ch * TCH:(ch + 1) * TCH],
                        )

            # --- persistent per-stream state ---
            # hs ping-pong: [carry | h(0..L-1)]; col 0 only ever holds the
            # running h carry (written at block end), cols 1..L the sweep's h.
            hs = [[singles.tile([128, BS, L + 1], F16, tag=f"hs{s}{i}",
                                name=f"hs{s}{i}") for i in range(2)]
                  for s in range(S)]
            # delta ping-pong: col 0 is always zero (carry delta).
            dlt = [[singles.tile([128, BS, L], F16, tag=f"dl{s}{i}",
                                 name=f"dl{s}{i}") for i in range(2)]
                   for s in range(S)]
            carry_c = [singles.tile([128, BS], F32, tag=f"cc{s}", name=f"cc{s}")
                       for s in range(S)]
            for s in range(S):
                nc.gpsimd.memset(dlt[s][0][:, :, 0], 0.0)
                nc.gpsimd.memset(dlt[s][1][:, :, 0], 0.0)

            out_r = out_h[:, :].rearrange("p (u t) -> p u t", u=U)

            # ---- per-stream item emission, streams staggered by one item
            # so xg bursts and last-sweeps of different streams spread
            # across rounds instead of clustering in-phase ----
            cpb = max(1, 512 // C)
            BH = BS // NHALF          # seqs per column group
            CH = C // NHALF           # cols per column group

            def emit_xg(s, blk):
                t0 = blk * L
                for g in range(4):
                    for k in range(2):
                        nc.tensor.matmul(
                            ps[s][:, g, :],
                            lhsT=wih_sb[:, k, g * 128:(g + 1) * 128],
                            rhs=xts[s][:, k, :, t0:t0 + L],
                            start=(k == 0 and g % cpb == 0), stop=False,
                            skip_group_check=True,
                        )
                    nc.tensor.matmul(
                        ps[s][:, g, :],
                        lhsT=bias_sb[:, g * 128:(g + 1) * 128],
                        rhs=ones_sb,
                        start=False, stop=False, skip_group_check=True,
                    )

            def emit_sweep_pre(s, blk, sw):
                t0 = blk * L
                last = sw == k_sweeps - 1
                adt = F32 if last else F16
                sfx = "32" if last else ""
                nb = 1 if last else 2
                ifo_t = work.tile([128, 4, C], adt, tag=f"ifo{s}{sfx}",
                                  bufs=nb, name=f"ifo{s}{sfx}")
                if SPLIT_SIG:
                    # chunks 0 (i) and 3 (g) first (they feed z), then {f, o}
                    nc.scalar.activation(out=ifo_t[:, 0::3, :],
                                         in_=ps[s][:, 0::3, :], func=sig)
                    nc.scalar.activation(out=ifo_t[:, 1:3, :],
                                         in_=ps[s][:, 1:3, :], func=sig)
                else:
                    nc.scalar.activation(out=ifo_t, in_=ps[s][:, :, :],
                                         func=sig)
                z_t = work.tile([128, C], adt, tag=f"z{s}{sfx}", bufs=nb,
                                name=f"z{s}{sfx}")
                c_t = work.tile([128, C], F32, tag=f"c{s}", bufs=2,
                                name=f"c{s}")
                for h in range(NHALF):
                    cs = slice(h * CH, (h + 1) * CH)
                    nc.vector.scalar_tensor_tensor(
                        out=z_t[:, cs], in0=ifo_t[:, 3, cs], scalar=0.5,
                        in1=ifo_t[:, 0, cs], op0=sub, op1=mult)
                    for u in range(h * BH, (h + 1) * BH):
                        nc.vector.tensor_tensor_scan(
                            out=c_t[:, u * L:(u + 1) * L],
                            data0=ifo_t[:, 1, u * L:(u + 1) * L],
                            data1=z_t[:, u * L:(u + 1) * L],
                            initial=carry_c[s][:, u:u + 1],
                            op0=mult, op1=add,
                        )
                return ifo_t, z_t, c_t

            def emit_sweep_post(s, blk, sw, pre):
                t0 = blk * L
                last = sw == k_sweeps - 1
                adt = F32 if last else F16
                sfx = "32" if last else ""
                nb = 1 if last else 2
                ifo_t, z_t, c_t = pre
                o_v = ifo_t[:, 2, :].rearrange("p (u t) -> p u t", u=BS)
                if sw == 0 and k_sweeps > 2:
                    # Sweep-0 feedback h tolerates a crude tanh: its error
                    # contracts ~rho^2 (~0.07) before the output, so use
                    # 2*clamp(c/2, +-0.5) on the DVE and skip the ScalarE
                    # tanh entirely (ScalarE is the bottleneck engine).
                    cl_t = work.tile([128, C], F16, tag=f"cl{s}", bufs=2,
                                     name=f"cl{s}")
                    nc.vector.tensor_scalar(
                        cl_t, c_t, 0.5, -0.5,
                        op0=mybir.AluOpType.min, op1=mybir.AluOpType.max)
                    cl_v = cl_t.rearrange("p (u t) -> p u t", u=BS)
                    hsN = hs[s][0]
                    nc.vector.scalar_tensor_tensor(
                        out=hsN[:, :, 1:L + 1], in0=cl_v, scalar=2.0,
                        in1=o_v, op0=mult, op1=mult)
                    rhs = hs[s][0][:, :, 0:L]
                    for g in range(4):
                        nc.tensor.matmul(
                            ps[s][:, g, :],
                            lhsT=whh_sb[:, g * 128:(g + 1) * 128],
                            rhs=rhs,
                            start=False, stop=False,
                            skip_group_check=True,
                        )
                    return
                tc_t = work.tile([128, C], adt, tag=f"tc{s}{sfx}", bufs=nb,
                                 name=f"tc{s}{sfx}")
                for h in range(NHALF):
                    cs = slice(h * CH, (h + 1) * CH)
                    nc.scalar.activation(out=tc_t[:, cs], in_=c_t[:, cs],
                                         func=tanh, scale=2.0)
                tc_v = tc_t.rearrange("p (u t) -> p u t", u=BS)
                if last:
                    out_t = work.tile([128, BS, L], F32, tag=f"out{s}",
                                      bufs=2, name=f"out{s}")
                    for h in range(NHALF):
                        us = slice(h * BH, (h + 1) * BH)
                        nc.vector.tensor_mul(out_t[:, us], o_v[:, us],
                                             tc_v[:, us])
                    u0 = s * BS
                    nc.sync.dma_start(
                        out=out_r[:, u0:u0 + BS, t0:t0 + L], in_=out_t)
                    if blk < NBLK - 1:
                        nc.gpsimd.tensor_copy(out=hs[s][0][:, :, 0],
                                              in_=out_t[:, :, L - 1])
                        nc.gpsimd.tensor_copy(
                            out=carry_c[s],
                            in_=c_t.rearrange(
                                "p (u t) -> p u t", u=BS)[:, :, L - 1])
                    return
                stop_all = sw == k_sweeps - 2
                for h in range(NHALF):
                    us = slice(h * BH, (h + 1) * BH)
                    hsN = hs[s][sw % 2]
                    nc.vector.tensor_mul(hsN[:, us, 1:L + 1], o_v[:, us],
                                         tc_v[:, us])
                    if sw > 0:
                        nc.vector.tensor_sub(
                            dlt[s][sw % 2][:, us, 1:L],
                            hs[s][sw % 2][:, us, 1:L],
                            hs[s][(sw + 1) % 2][:, us, 1:L])
                    rhs = (hs[s][0][:, us, 0:L] if sw == 0
                           else dlt[s][sw % 2][:, us, 0:L])
                    for g in range(4):
                        nc.tensor.matmul(
                            ps[s][:, g, h * CH:(h + 1) * CH],
                            lhsT=whh_sb[:, g * 128:(g + 1) * 128],
                            rhs=rhs,
                            start=False,
                            stop=(stop_all and g == 3 and h == NHALF - 1),
                            skip_group_check=True,
                        )

            total_items = NBLK * (k_sweeps + 1)
            for _rep in range(reps):
              for s in range(S):
                nc.vector.memset(carry_c[s], 0.0)
                nc.gpsimd.memset(hs[s][0][:, :, 0], 0.0)
              for t in range(total_items + (S - 1) * STAGGER):
                  pres = {}
                  for s in range(S):
                      idx = t - s * STAGGER
                      if not (0 <= idx < total_items):
                          continue
                      blk, ph = divmod(idx, k_sweeps + 1)
                      if ph == 0:
                          emit_xg(s, blk)
                      else:
                          pres[s] = (blk, ph - 1,
                                     emit_sweep_pre(s, blk, ph - 1))
                  for s, (blk, sw, pre) in pres.items():
                      emit_sweep_post(s, blk, sw, pre)

    if not nc.is_finalized():
        nc.finalize()
    return nc


def _get_nc(reps=1):
    key = f"nc{reps}"
    if key not in _NC_CACHE:
        _NC_CACHE[key] = _build_nc(reps=reps)
    return _NC_CACHE[key]


def _flip_padded(x, lengths):
    t = np.arange(x.shape[1])[None, :]
    Ln = lengths[:, None].astype(np.int64)
    idx = np.where(t < Ln, Ln - 1 - t, t)
    return np.take_along_axis(x, idx[:, :, None], axis=1)


def _pack_weights(W_ih, W_hh, b_ih, b_hh):
    # chunk order (i, f, o, g); the g chunk is pre-scaled by 2 because the
    # kernel computes tanh(g) as 2*sigmoid(2g) - 1 inside the fused sigmoid
    # instruction.
    Wi = W_ih.reshape(4, H, I)[PERM].copy()             # [4,128,256]
    Wi[3] *= 2.0
    wih = np.ascontiguousarray(
        Wi.transpose(2, 0, 1).reshape(2, 128, G4)).astype(np.float16)
    Wh = W_hh.reshape(4, H, H)[PERM].copy()             # [4,128,128]
    Wh[3] *= 2.0
    whh = np.ascontiguousarray(
        Wh.transpose(2, 0, 1).reshape(128, G4)).astype(np.float16)
    b4 = (b_ih + b_hh).reshape(4, H)[PERM].copy()
    b4[3] *= 2.0
    b = b4.reshape(1, G4).astype(np.float16)
    return wih, whh, np.ascontiguousarray(b)


def _pack_x(x_shard):
    # [U, T, I] -> [2, 128, U*T] with cols (u, t) u-major
    a = x_shard.transpose(2, 0, 1).reshape(2, 128, U * T)
    return np.ascontiguousarray(a).astype(np.float16)


def _run(inputs, trace=False):
    x = np.asarray(inputs["x"], np.float32)
    lengths = np.asarray(inputs["lengths"])
    Wf_ih = np.asarray(inputs["Wf_ih"], np.float32)
    Wf_hh = np.asarray(inputs["Wf_hh"], np.float32)
    bf_ih = np.asarray(inputs["bf_ih"], np.float32)
    bf_hh = np.asarray(inputs["bf_hh"], np.float32)
    Wb_ih = np.asarray(inputs["Wb_ih"], np.float32)
    Wb_hh = np.asarray(inputs["Wb_hh"], np.float32)
    bb_ih = np.asarray(inputs["bb_ih"], np.float32)
    bb_hh = np.asarray(inputs["bb_hh"], np.float32)

    x_rev = _flip_padded(x, lengths)
    wf = _pack_weights(Wf_ih, Wf_hh, bf_ih, bf_hh)
    wb = _pack_weights(Wb_ih, Wb_hh, bb_ih, bb_hh)

    in_maps = []
    for c in range(NCORES):
        if c < 4:
            xs = x[c * U:(c + 1) * U]
            wih, whh, b = wf
        else:
            xs = x_rev[(c - 4) * U:(c - 3) * U]
            wih, whh, b = wb
        in_maps.append({
            "xt": _pack_x(xs),
            "wih": wih,
            "whh": whh,
            "bias": b,
        })

    nc = _get_nc()
    res = run_bass_kernel_spmd(nc, in_maps, core_ids=list(range(NCORES)),
                               trace=trace)
    halves = []
    for c in range(NCORES):
        o = res.results[c]["out"].reshape(128, U, T).transpose(1, 2, 0)
        halves.append(o)
    fwd = np.concatenate(halves[0:4], axis=0)   # [32, T, 128]
    bwd = np.concatenate(halves[4:8], axis=0)   # [32, T, 128]
    out = np.concatenate([fwd, bwd], axis=-1).astype(np.float32)
    return out, res.exec_time_ns


def kernel(**inputs):
    out, _ = _run(inputs, trace=False)
    return out



# revision 4
# speedup vs baseline: 1.2769x; 1.2769x over previous
"""Bidirectional LSTM (B=32, T=2048, I=256, H=128/dir) for 8 Trainium2 cores.

Sharding: data-parallel over (batch, direction) - cores 0-3 run the forward
LSTM over 8 batch rows each, cores 4-7 run the backward LSTM over the
host-flipped sequences.

Per core the nonlinear recurrence is solved with block fixed-point
iteration: time is processed in blocks of L=128 steps; within a block,
K_SWEEPS sweeps each recompute all gates with one batched matmul feedback,
apply sigmoid over the whole block at once, run the c-recurrence with the
hardware tensor_tensor_scan, and recompute h.  Error contracts ~3.7x per
sweep; K=3 with the sweep-0 clamp-tanh measures rel-l2 ~7e-3 vs the fp32
reference, under the 2e-2 gate with ~3x margin.

Engine-level design (ScalarE is the bottleneck: 5C activation elements per
block-stream):
 - S=2 streams of BS=4 sequences (C=512 gate columns each); each stream's
   gate accumulator [128, 4, C] f32 fills 4 PSUM banks (all 8 used).  Fewer,
   larger instructions: per-instruction fixed overhead (~0.2-0.4us on
   ScalarE/DVE) was measured to dominate over smaller-instruction overlap
   (S=8 probe ran 2.3x slower than S=4; S=2 beats S=4).
 - Streams are emitted in rounds with a one-item stagger so each engine's
   in-order queue interleaves the two streams: stream B's sigmoid sits
   between stream A's sigmoid and A's tanh, covering the DVE scan latency.
 - Feedback matmuls in delta form: gates += W_hh @ (h_new - h_old), 4
   matmuls per sweep.  Sweep 0's "delta" is the h tile itself ([carry |
   h_0] vs the implicit all-zero initial guess).  Delta tiles keep column
   0 == 0 (the carry never changes within a block).
 - Everything fp16 except the PSUM accumulators and the c-scan internal
   state (hardware keeps scan state fp32 regardless of operand dtype) and
   the fp32 carry: x, W in fp16 (fp16 matmul is full PE rate); gates,
   z, c, tanh, h, deltas, output all fp16 (~5e-4 rel rounding, negligible
   against the 7e-3 iteration error).  Output DMA'd as fp16 (half the
   bytes), widened to fp32 on the host.
 - z fused to one DVE op: z/2 = (sigmoid(2g) - 0.5) * sigmoid(i); the
   c-scan is linear in z so it just produces c/2, undone for free by the
   tanh's input scale=2.  The g rows of W_ih/W_hh/bias are pre-scaled by
   2 on the host so one batched sigmoid covers all four gate chunks.
 - Whole x preloaded to SBUF in 4 chunked DMAs (contiguous >=1KB runs).
 - Sweep 0's feedback tanh runs as 2*clamp(c/2, +-0.5) on the DVE (fused
   with the h multiply via scalar_tensor_tensor): its error contracts
   ~rho^2 before the output, and it removes tanh work from ScalarE, the
   bottleneck engine.
 - The non-feedback element-wise products (final h = sigmoid(o)*tanh(c)
   and the delta subtraction) run on the otherwise-idle GPSIMD engine,
   keeping DVE (scans + z) off the critical balance.

_build_nc(reps=R) emits R back-to-back repetitions of the kernel (with
per-rep carry resets, so the output stays exact) - used by test.py to
amortize the per-dispatch axon overhead when timing; the graded kernel()
path uses reps=1.
"""

import numpy as np

import concourse.bass as bass
import concourse.bacc as bacc
import concourse.tile as tile
from concourse import mybir
from concourse.bass_utils import run_bass_kernel_spmd

# Problem shapes (hardcoded per contract)
B, T, I, HS = 32, 2048, 256, 256
H = 128          # per-direction hidden
G4 = 4 * H       # 512 stacked gates
NCORES = 8
U = 8            # sequences per core
S = 2            # independent streams per core (pipelining)
BS = U // S      # sequences per stream (4)
L = 128          # time-block length
NBLK = T // L
K_SWEEPS = 3
C = BS * L       # gate columns per stream-block (512)
NHALF = 1        # column groups per stream for intra-stream pipelining
STAGGER = 1      # per-stream item offset
SPLIT_SIG = False  # sigmoid in 2 chunk-group instructions
NO_OUT_DMA = False  # timing-probe knob: skip the output DMA
GP_DELTA = True     # delta-sub on gpsimd
GP_OUT = True       # final h multiply on gpsimd

# gate chunk order inside the 4*H dim: (i, f, o, g); reference order is (i, f, g, o)
PERM = [0, 1, 3, 2]

F32 = mybir.dt.float32
F16 = mybir.dt.float16

_NC_CACHE = {}


def _build_nc(k_sweeps=K_SWEEPS, reps=1):
    nc = bacc.Bacc()
    xt_h = nc.dram_tensor("xt", [2, 128, U * T], F16, kind="ExternalInput")
    wih_h = nc.dram_tensor("wih", [2, 128, G4], F16, kind="ExternalInput")
    whh_h = nc.dram_tensor("whh", [128, G4], F16, kind="ExternalInput")
    bias_h = nc.dram_tensor("bias", [1, G4], F16, kind="ExternalInput")
    out_h = nc.dram_tensor("out", [128, U * T], F16, kind="ExternalOutput")

    sig = mybir.ActivationFunctionType.Sigmoid
    tanh = mybir.ActivationFunctionType.Tanh
    mult = mybir.AluOpType.mult
    add = mybir.AluOpType.add
    sub = mybir.AluOpType.subtract

    with tile.TileContext(nc) as tc:
        with (
            tc.tile_pool(name="singles", bufs=1) as singles,
            tc.tile_pool(name="work", bufs=2) as work,
            tc.tile_pool(name="psum", bufs=1, space="PSUM") as psump,
        ):
            # --- weights / constants ---
            wih_sb = singles.tile([128, 2, G4], F16, tag="wih")
            nc.sync.dma_start(out=wih_sb, in_=wih_h[:, :, :].transpose([1, 0, 2]))
            whh_sb = singles.tile([128, G4], F16, tag="whh")
            nc.sync.dma_start(out=whh_sb, in_=whh_h[:, :])
            bias_sb = singles.tile([1, G4], F16, tag="bias")
            nc.sync.dma_start(out=bias_sb, in_=bias_h[:, :])
            ones_sb = singles.tile([1, C], F16, tag="ones")
            nc.vector.memset(ones_sb.bitcast(mybir.dt.uint16), 0x3C00)

            # gate accumulators: S streams x (8/S) banks = all 8 PSUM banks
            ps = [psump.tile([128, 4, C], F32, tag=f"ps{s}", name=f"ps{s}")
                  for s in range(S)]

            # Warm-up matmuls: consume every lhsT weight tile once so later
            # matmuls inherit the weight-DMA dependencies via PE program
            # order instead of carrying their own sync waits (the LDW
            # instruction has very few wait slots).
            nc.tensor.matmul(ps[0][:, 0, :], lhsT=whh_sb[:, 0:128],
                             rhs=whh_sb[:, 0:C], start=True, stop=True,
                             skip_group_check=True)
            nc.tensor.matmul(ps[0][:, 0, :], lhsT=wih_sb[:, 0, 0:128],
                             rhs=wih_sb[:, 1, 0:C], start=True, stop=True,
                             skip_group_check=True)
            nc.tensor.matmul(ps[0][:, 0, :], lhsT=bias_sb[:, 0:128],
                             rhs=ones_sb, start=True, stop=True,
                             skip_group_check=True)

            # --- x preload: [128, 2(k), BS, T] per stream, 4 t-chunks ---
            xt_r = xt_h[:, :, :].transpose([1, 0, 2]).rearrange(
                "p k (u t) -> p k u t", u=U)
            xts = [singles.tile([128, 2, BS, T], F16, tag=f"xt{s}",
                                name=f"xt{s}") for s in range(S)]
            NCHUNK = 4
            TCH = T // NCHUNK
            for ch in range(NCHUNK):
                for s in range(S):
                    u0 = s * BS
                    for k in range(2):
                        nc.sync.dma_start(
                            out=xts[s][:, k, :, ch * TCH:(ch + 1) * TCH],
                            in_=xt_r[:, k, u0:u0 + BS, ch * TCH:(ch + 1) * TCH],
                        )

            # --- persistent per-stream state ---
            # hs ping-pong: [carry | h(0..L-1)]; col 0 only ever holds the
            # running h carry (written at block end), cols 1..L the sweep's h.
            hs = [[singles.tile([128, BS, L + 1], F16, tag=f"hs{s}{i}",
                                name=f"hs{s}{i}") for i in range(2)]
                  for s in range(S)]
            # delta ping-pong: col 0 is always zero (carry delta).
            dlt = [[singles.tile([128, BS, L], F16, tag=f"dl{s}{i}",
                                 name=f"dl{s}{i}") for i in range(2)]
                   for s in range(S)]
            carry_c = [singles.tile([128, BS], F32, tag=f"cc{s}", name=f"cc{s}")
                       for s in range(S)]
            for s in range(S):
                nc.gpsimd.memset(dlt[s][0][:, :, 0], 0.0)
                nc.gpsimd.memset(dlt[s][1][:, :, 0], 0.0)

            out_r = out_h[:, :].rearrange("p (u t) -> p u t", u=U)

            # ---- per-stream item emission, streams staggered by one item
            # so xg bursts and last-sweeps of different streams spread
            # across rounds instead of clustering in-phase ----
            cpb = max(1, 512 // C)
            BH = BS // NHALF          # seqs per column group
            CH = C // NHALF           # cols per column group

            def emit_xg(s, blk):
                t0 = blk * L
                for g in range(4):
                    for k in range(2):
                        nc.tensor.matmul(
                            ps[s][:, g, :],
                            lhsT=wih_sb[:, k, g * 128:(g + 1) * 128],
                            rhs=xts[s][:, k, :, t0:t0 + L],
                            start=(k == 0 and g % cpb == 0), stop=False,
                            skip_group_check=True,
                        )
                    nc.tensor.matmul(
                        ps[s][:, g, :],
                        lhsT=bias_sb[:, g * 128:(g + 1) * 128],
                        rhs=ones_sb,
                        start=False, stop=False, skip_group_check=True,
                    )

            def emit_sweep_pre(s, blk, sw):
                last = sw == k_sweeps - 1
                ifo_t = work.tile([128, 4, C], F16, tag=f"ifo{s}", bufs=2,
                                  name=f"ifo{s}")
                if SPLIT_SIG:
                    # chunks 0 (i) and 3 (g) first (they feed z), then {f, o}
                    nc.scalar.activation(out=ifo_t[:, 0::3, :],
                                         in_=ps[s][:, 0::3, :], func=sig)
                    nc.scalar.activation(out=ifo_t[:, 1:3, :],
                                         in_=ps[s][:, 1:3, :], func=sig)
                else:
                    nc.scalar.activation(out=ifo_t, in_=ps[s][:, :, :],
                                         func=sig)
                z_t = work.tile([128, C], F16, tag=f"z{s}", bufs=2,
                                name=f"z{s}")
                c_t = work.tile([128, C], F16, tag=f"c{s}", bufs=2,
                                name=f"c{s}")
                for h in range(NHALF):
                    cs = slice(h * CH, (h + 1) * CH)
                    nc.vector.scalar_tensor_tensor(
                        out=z_t[:, cs], in0=ifo_t[:, 3, cs], scalar=0.5,
                        in1=ifo_t[:, 0, cs], op0=sub, op1=mult)
                    for u in range(h * BH, (h + 1) * BH):
                        nc.vector.tensor_tensor_scan(
                            out=c_t[:, u * L:(u + 1) * L],
                            data0=ifo_t[:, 1, u * L:(u + 1) * L],
                            data1=z_t[:, u * L:(u + 1) * L],
                            initial=carry_c[s][:, u:u + 1],
                            op0=mult, op1=add,
                        )
                return ifo_t, z_t, c_t

            def emit_sweep_post(s, blk, sw, pre):
                t0 = blk * L
                last = sw == k_sweeps - 1
                ifo_t, z_t, c_t = pre
                o_v = ifo_t[:, 2, :].rearrange("p (u t) -> p u t", u=BS)
                if sw == 0 and k_sweeps > 2:
                    # Sweep-0 feedback h tolerates a crude tanh: its error
                    # contracts ~rho^2 (~0.07) before the output, so use
                    # 2*clamp(c/2, +-0.5) on the DVE and skip the ScalarE
                    # tanh entirely (ScalarE is the bottleneck engine).
                    cl_t = work.tile([128, C], F16, tag=f"cl{s}", bufs=2,
                                     name=f"cl{s}")
                    nc.vector.tensor_scalar(
                        cl_t, c_t, 0.5, -0.5,
                        op0=mybir.AluOpType.min, op1=mybir.AluOpType.max)
                    cl_v = cl_t.rearrange("p (u t) -> p u t", u=BS)
                    hsN = hs[s][0]
                    nc.vector.scalar_tensor_tensor(
                        out=hsN[:, :, 1:L + 1], in0=cl_v, scalar=2.0,
                        in1=o_v, op0=mult, op1=mult)
                    rhs = hs[s][0][:, :, 0:L]
                    for g in range(4):
                        nc.tensor.matmul(
                            ps[s][:, g, :],
                            lhsT=whh_sb[:, g * 128:(g + 1) * 128],
                            rhs=rhs,
                            start=False, stop=False,
                            skip_group_check=True,
                        )
                    return
                tc_t = work.tile([128, C], F16, tag=f"tc{s}", bufs=2,
                                 name=f"tc{s}")
                for h in range(NHALF):
                    cs = slice(h * CH, (h + 1) * CH)
                    nc.scalar.activation(out=tc_t[:, cs], in_=c_t[:, cs],
                                         func=tanh, scale=2.0)
                tc_v = tc_t.rearrange("p (u t) -> p u t", u=BS)
                if last:
                    out_t = work.tile([128, BS, L], F16, tag=f"out{s}",
                                      bufs=2, name=f"out{s}")
                    eng_out = nc.gpsimd if GP_OUT else nc.vector
                    for h in range(NHALF):
                        us = slice(h * BH, (h + 1) * BH)
                        eng_out.tensor_mul(out_t[:, us], o_v[:, us],
                                           tc_v[:, us])
                    u0 = s * BS
                    if not NO_OUT_DMA:
                        nc.sync.dma_start(
                            out=out_r[:, u0:u0 + BS, t0:t0 + L], in_=out_t)
                    if blk < NBLK - 1:
                        nc.gpsimd.tensor_copy(out=hs[s][0][:, :, 0],
                                              in_=out_t[:, :, L - 1])
                        nc.gpsimd.tensor_copy(
                            out=carry_c[s],
                            in_=c_t.rearrange(
                                "p (u t) -> p u t", u=BS)[:, :, L - 1])
                    return
                stop_all = sw == k_sweeps - 2
                eng_d = nc.gpsimd if GP_DELTA else nc.vector
                for h in range(NHALF):
                    us = slice(h * BH, (h + 1) * BH)
                    hsN = hs[s][sw % 2]
                    nc.vector.tensor_mul(hsN[:, us, 1:L + 1], o_v[:, us],
                                         tc_v[:, us])
                    if sw > 0:
                        eng_d.tensor_sub(
                            dlt[s][sw % 2][:, us, 1:L],
                            hs[s][sw % 2][:, us, 1:L],
                            hs[s][(sw + 1) % 2][:, us, 1:L])
                    rhs = (hs[s][0][:, us, 0:L] if sw == 0
                           else dlt[s][sw % 2][:, us, 0:L])
                    for g in range(4):
                        nc.tensor.matmul(
                            ps[s][:, g, h * CH:(h + 1) * CH],
                            lhsT=whh_sb[:, g * 128:(g + 1) * 128],
                            rhs=rhs,
                            start=False,
                            stop=(stop_all and g == 3 and h == NHALF - 1),
                            skip_group_check=True,
                        )

            total_items = NBLK * (k_sweeps + 1)
            for _rep in range(reps):
              for s in range(S):
                nc.vector.memset(carry_c[s], 0.0)
                nc.gpsimd.memset(hs[s][0][:, :, 0], 0.0)
              for t in range(total_items + (S - 1) * STAGGER):
                  pres = {}
                  for s in range(S):
                      idx = t - s * STAGGER
                      if not (0 <= idx < total_items):
                          continue
                      blk, ph = divmod(idx, k_sweeps + 1)
                      if ph == 0:
                          emit_xg(s, blk)
                      else:
                          pres[s] = (blk, ph - 1,
                                     emit_sweep_pre(s, blk, ph - 1))
                  for s, (blk, sw, pre) in pres.items():
                      emit_sweep_post(s, blk, sw, pre)

    if not nc.is_finalized():
        nc.finalize()
    return nc


def _get_nc(reps=1):
    key = f"nc{reps}"
    if key not in _NC_CACHE:
        _NC_CACHE[key] = _build_nc(reps=reps)
    return _NC_CACHE[key]


def _flip_padded(x, lengths):
    t = np.arange(x.shape[1])[None, :]
    Ln = lengths[:, None].astype(np.int64)
    idx = np.where(t < Ln, Ln - 1 - t, t)
    return np.take_along_axis(x, idx[:, :, None], axis=1)


def _pack_weights(W_ih, W_hh, b_ih, b_hh):
    # chunk order (i, f, o, g); the g chunk is pre-scaled by 2 because the
    # kernel computes tanh(g) as 2*sigmoid(2g) - 1 inside the fused sigmoid
    # instruction.
    Wi = W_ih.reshape(4, H, I)[PERM].copy()             # [4,128,256]
    Wi[3] *= 2.0
    wih = np.ascontiguousarray(
        Wi.transpose(2, 0, 1).reshape(2, 128, G4)).astype(np.float16)
    Wh = W_hh.reshape(4, H, H)[PERM].copy()             # [4,128,128]
    Wh[3] *= 2.0
    whh = np.ascontiguousarray(
        Wh.transpose(2, 0, 1).reshape(128, G4)).astype(np.float16)
    b4 = (b_ih + b_hh).reshape(4, H)[PERM].copy()
    b4[3] *= 2.0
    b = b4.reshape(1, G4).astype(np.float16)
    return wih, whh, np.ascontiguousarray(b)


def _pack_x(x_shard):
    # [U, T, I] -> [2, 128, U*T] with cols (u, t) u-major
    a = x_shard.transpose(2, 0, 1).reshape(2, 128, U * T)
    return np.ascontiguousarray(a).astype(np.float16)


def _run(inputs, trace=False):
    x = np.asarray(inputs["x"], np.float32)
    lengths = np.asarray(inputs["lengths"])
    Wf_ih = np.asarray(inputs["Wf_ih"], np.float32)
    Wf_hh = np.asarray(inputs["Wf_hh"], np.float32)
    bf_ih = np.asarray(inputs["bf_ih"], np.float32)
    bf_hh = np.asarray(inputs["bf_hh"], np.float32)
    Wb_ih = np.asarray(inputs["Wb_ih"], np.float32)
    Wb_hh = np.asarray(inputs["Wb_hh"], np.float32)
    bb_ih = np.asarray(inputs["bb_ih"], np.float32)
    bb_hh = np.asarray(inputs["bb_hh"], np.float32)

    x_rev = _flip_padded(x, lengths)
    wf = _pack_weights(Wf_ih, Wf_hh, bf_ih, bf_hh)
    wb = _pack_weights(Wb_ih, Wb_hh, bb_ih, bb_hh)

    in_maps = []
    for c in range(NCORES):
        if c < 4:
            xs = x[c * U:(c + 1) * U]
            wih, whh, b = wf
        else:
            xs = x_rev[(c - 4) * U:(c - 3) * U]
            wih, whh, b = wb
        in_maps.append({
            "xt": _pack_x(xs),
            "wih": wih,
            "whh": whh,
            "bias": b,
        })

    nc = _get_nc()
    res = run_bass_kernel_spmd(nc, in_maps, core_ids=list(range(NCORES)),
                               trace=trace)
    halves = []
    for c in range(NCORES):
        o = res.results[c]["out"].reshape(128, U, T).transpose(1, 2, 0)
        halves.append(o.astype(np.float32))
    fwd = np.concatenate(halves[0:4], axis=0)   # [32, T, 128]
    bwd = np.concatenate(halves[4:8], axis=0)   # [32, T, 128]
    out = np.concatenate([fwd, bwd], axis=-1).astype(np.float32)
    return out, res.exec_time_ns


def kernel(**inputs):
    out, _ = _run(inputs, trace=False)
    return out


# revision 16
# speedup vs baseline: 1.6388x; 1.2834x over previous
"""Bidirectional LSTM (B=32, T=2048, I=256, H=128/dir) for 8 Trainium2 cores.

Sharding: data-parallel over (batch, direction) - cores 0-3 run the forward
LSTM over 8 batch rows each, cores 4-7 run the backward LSTM over the
host-flipped sequences.

Per core the nonlinear recurrence is solved with block fixed-point
iteration: time is processed in blocks of L=128 steps; within a block,
K_SWEEPS sweeps each recompute all gates with one batched matmul feedback,
apply sigmoid over the whole block at once, run the c-recurrence with the
hardware tensor_tensor_scan, and recompute h.  Error contracts ~3.7x per
sweep; K=3 with the sweep-0 clamp-tanh measures rel-l2 ~7e-3 vs the fp32
reference, under the 2e-2 gate with ~3x margin.

Engine-level design (ScalarE is the bottleneck: 5C activation elements per
block-stream):
 - S=2 streams of BS=4 sequences (C=512 gate columns each); each stream's
   gate accumulator [128, 4, C] f32 fills 4 PSUM banks (all 8 used).  Fewer,
   larger instructions: per-instruction fixed overhead (~0.2-0.4us on
   ScalarE/DVE) was measured to dominate over smaller-instruction overlap
   (S=8 probe ran 2.3x slower than S=4; S=2 beats S=4).
 - Streams are emitted in rounds with a one-item stagger so each engine's
   in-order queue interleaves the two streams: stream B's sigmoid sits
   between stream A's sigmoid and A's tanh, covering the DVE scan latency.
 - Feedback matmuls in delta form: gates += W_hh @ (h_new - h_old), 4
   matmuls per sweep.  Sweep 0's "delta" is the h tile itself ([carry |
   h_0] vs the implicit all-zero initial guess).  Delta tiles keep column
   0 == 0 (the carry never changes within a block).
 - Everything fp16 except the PSUM accumulators and the c-scan internal
   state (hardware keeps scan state fp32 regardless of operand dtype) and
   the fp32 carry: x, W in fp16 (fp16 matmul is full PE rate); gates,
   z, c, tanh, h, deltas, output all fp16 (~5e-4 rel rounding, negligible
   against the 7e-3 iteration error).  Output DMA'd as fp16 (half the
   bytes), widened to fp32 on the host.
 - z fused to one DVE op: z/2 = (sigmoid(2g) - 0.5) * sigmoid(i); the
   c-scan is linear in z so it just produces c/2, undone for free by the
   tanh's input scale=2.  The g rows of W_ih/W_hh/bias are pre-scaled by
   2 on the host so one batched sigmoid covers all four gate chunks.
 - Whole x preloaded to SBUF in 4 chunked DMAs (contiguous >=1KB runs).
 - Sweep 0's feedback tanh runs as 2*clamp(c/2, +-0.5) on the DVE (fused
   with the h multiply via scalar_tensor_tensor): its error contracts
   ~rho^2 before the output, and it removes tanh work from ScalarE, the
   bottleneck engine.
 - The non-feedback element-wise products (final h = sigmoid(o)*tanh(c)
   and the delta subtraction) run on the otherwise-idle GPSIMD engine,
   keeping DVE (scans + z) off the critical balance.

_build_nc(reps=R) emits R back-to-back repetitions of the kernel (with
per-rep carry resets, so the output stays exact) - used by test.py to
amortize the per-dispatch axon overhead when timing; the graded kernel()
path uses reps=1.
"""

import numpy as np

import concourse.bass as bass
import concourse.bacc as bacc
import concourse.tile as tile
from concourse import mybir
from concourse.bass_utils import run_bass_kernel_spmd

# Problem shapes (hardcoded per contract)
B, T, I, HS = 32, 2048, 256, 256
H = 128          # per-direction hidden
G4 = 4 * H       # 512 stacked gates
NCORES = 8
U = 8            # sequences per core
S = 4            # independent streams per core (pipelining)
BS = U // S      # sequences per stream (2)
L = 128          # time-block length
NBLK = T // L
K_SWEEPS = 3
C = BS * L       # gate columns per stream-block (256)
NHALF = 1        # column groups per stream for intra-stream pipelining
STAGGER = 2      # per-stream item offset
SPLIT_SIG = False  # sigmoid in 2 chunk-group instructions
NO_OUT_DMA = False  # timing-probe knob: skip the output DMA
GP_DELTA = False    # delta-sub on gpsimd (on the feedback critical path)
GP_OUT = True       # final h multiply on gpsimd
PE_DELTA = True     # middle-sweep feedback as +W*h_new / -W*h_old matmul
                    # pairs (PE has headroom) instead of an explicit
                    # h_new-h_old DVE op on the feedback critical path
OUT_F16 = True      # output DRAM tensor fp16 (host widens to fp32)

# gate chunk order inside the 4*H dim: (i, f, o, g); reference order is (i, f, g, o)
PERM = [0, 1, 3, 2]

F32 = mybir.dt.float32
F16 = mybir.dt.float16

_NC_CACHE = {}


def _build_nc(k_sweeps=K_SWEEPS, reps=1):
    nc = bacc.Bacc()
    xt_h = nc.dram_tensor("xt", [2, 128, U * T], F16, kind="ExternalInput")
    wih_h = nc.dram_tensor("wih", [2, 128, G4], F16, kind="ExternalInput")
    whh_h = nc.dram_tensor("whh", [128, G4], F16, kind="ExternalInput")
    bias_h = nc.dram_tensor("bias", [1, G4], F16, kind="ExternalInput")
    out_dt = F16 if OUT_F16 else F32
    out_h = nc.dram_tensor("out", [128, U * T], out_dt, kind="ExternalOutput")

    sig = mybir.ActivationFunctionType.Sigmoid
    tanh = mybir.ActivationFunctionType.Tanh
    mult = mybir.AluOpType.mult
    add = mybir.AluOpType.add
    sub = mybir.AluOpType.subtract

    with tile.TileContext(nc) as tc:
        with (
            tc.tile_pool(name="singles", bufs=1) as singles,
            tc.tile_pool(name="work", bufs=2) as work,
            tc.tile_pool(name="psum", bufs=1, space="PSUM") as psump,
        ):
            # --- weights / constants ---
            wih_sb = singles.tile([128, 2, G4], F16, tag="wih")
            nc.sync.dma_start(out=wih_sb, in_=wih_h[:, :, :].transpose([1, 0, 2]))
            whh_sb = singles.tile([128, G4], F16, tag="whh")
            nc.sync.dma_start(out=whh_sb, in_=whh_h[:, :])
            whh_ng = None
            if PE_DELTA:
                # negated W_hh for the -W*h_old half of middle-sweep
                # feedback pairs (PSUM accumulation has no subtract mode)
                whh_ng = singles.tile([128, G4], F16, tag="whhn")
                nc.vector.tensor_scalar_mul(whh_ng, whh_sb, -1.0)
            bias_sb = singles.tile([1, G4], F16, tag="bias")
            nc.sync.dma_start(out=bias_sb, in_=bias_h[:, :])
            ones_sb = singles.tile([1, C], F16, tag="ones")
            nc.vector.memset(ones_sb.bitcast(mybir.dt.uint16), 0x3C00)

            # gate accumulators: S streams x (8/S) banks = all 8 PSUM banks
            ps = [psump.tile([128, 4, C], F32, tag=f"ps{s}", name=f"ps{s}")
                  for s in range(S)]

            # Warm-up matmuls: consume every lhsT weight tile once so later
            # matmuls inherit the weight-DMA dependencies via PE program
            # order instead of carrying their own sync waits (the LDW
            # instruction has very few wait slots).
            nc.tensor.matmul(ps[0][:, 0, :], lhsT=whh_sb[:, 0:128],
                             rhs=whh_sb[:, 0:C], start=True, stop=True,
                             skip_group_check=True)
            nc.tensor.matmul(ps[0][:, 0, :], lhsT=wih_sb[:, 0, 0:128],
                             rhs=wih_sb[:, 1, 0:C], start=True, stop=True,
                             skip_group_check=True)
            nc.tensor.matmul(ps[0][:, 0, :], lhsT=bias_sb[:, 0:128],
                             rhs=ones_sb, start=True, stop=True,
                             skip_group_check=True)

            # --- x preload: [128, 2(k), BS, T] per stream, 4 t-chunks ---
            xt_r = xt_h[:, :, :].transpose([1, 0, 2]).rearrange(
                "p k (u t) -> p k u t", u=U)
            xts = [singles.tile([128, 2, BS, T], F16, tag=f"xt{s}",
                                name=f"xt{s}") for s in range(S)]
            # first chunk covers just block 0 so compute starts right away
            # in the graded reps=1 path; the rest in big chunks
            bounds = [0, L, T // 4, T // 2, 3 * T // 4, T]
            for ch in range(len(bounds) - 1):
                c0, c1 = bounds[ch], bounds[ch + 1]
                for s in range(S):
                    u0 = s * BS
                    for k in range(2):
                        nc.sync.dma_start(
                            out=xts[s][:, k, :, c0:c1],
                            in_=xt_r[:, k, u0:u0 + BS, c0:c1],
                        )

            # --- persistent per-stream state ---
            # hs ping-pong: [carry | h(0..L-1)]; col 0 only ever holds the
            # running h carry (written at block end), cols 1..L the sweep's h.
            hs = [[singles.tile([128, BS, L + 1], F16, tag=f"hs{s}{i}",
                                name=f"hs{s}{i}") for i in range(2)]
                  for s in range(S)]
            # delta ping-pong: col 0 is always zero (carry delta).
            if not PE_DELTA:
                dlt = [[singles.tile([128, BS, L], F16, tag=f"dl{s}{i}",
                                     name=f"dl{s}{i}") for i in range(2)]
                       for s in range(S)]
                for s in range(S):
                    nc.gpsimd.memset(dlt[s][0][:, :, 0], 0.0)
                    nc.gpsimd.memset(dlt[s][1][:, :, 0], 0.0)
            carry_c = [singles.tile([128, BS], F32, tag=f"cc{s}", name=f"cc{s}")
                       for s in range(S)]

            out_r = out_h[:, :].rearrange("p (u t) -> p u t", u=U)

            # ---- per-stream item emission, streams staggered by one item
            # so xg bursts and last-sweeps of different streams spread
            # across rounds instead of clustering in-phase ----
            cpb = max(1, 512 // C)
            BH = BS // NHALF          # seqs per column group
            CH = C // NHALF           # cols per column group

            def emit_xg(s, blk):
                t0 = blk * L
                for g in range(4):
                    for k in range(2):
                        nc.tensor.matmul(
                            ps[s][:, g, :],
                            lhsT=wih_sb[:, k, g * 128:(g + 1) * 128],
                            rhs=xts[s][:, k, :, t0:t0 + L],
                            start=(k == 0 and g % cpb == 0), stop=False,
                            skip_group_check=True,
                        )
                    nc.tensor.matmul(
                        ps[s][:, g, :],
                        lhsT=bias_sb[:, g * 128:(g + 1) * 128],
                        rhs=ones_sb,
                        start=False, stop=False, skip_group_check=True,
                    )

            def emit_sweep_pre(s, blk, sw):
                last = sw == k_sweeps - 1
                ifo_t = work.tile([128, 4, C], F16, tag=f"ifo{s}", bufs=2,
                                  name=f"ifo{s}")
                if SPLIT_SIG:
                    # chunks 0 (i) and 3 (g) first (they feed z), then {f, o}
                    nc.scalar.activation(out=ifo_t[:, 0::3, :],
                                         in_=ps[s][:, 0::3, :], func=sig)
                    nc.scalar.activation(out=ifo_t[:, 1:3, :],
                                         in_=ps[s][:, 1:3, :], func=sig)
                else:
                    nc.scalar.activation(out=ifo_t, in_=ps[s][:, :, :],
                                         func=sig)
                if PE_DELTA and 0 < sw < k_sweeps - 1:
                    # -W*h_old half of the feedback pair: depends only on
                    # the sigmoid's PSUM read just above and the previous
                    # sweep's h, so the PE gets a head start while the DVE
                    # leg (z/scan/h) of this sweep still runs.  Column 0 of
                    # both hs tiles holds the same carry, so the +/- pair
                    # cancels it exactly.
                    rhs_old = hs[s][(sw + 1) % 2][:, :, 0:L]
                    for g in range(4):
                        nc.tensor.matmul(
                            ps[s][:, g, :],
                            lhsT=whh_ng[:, g * 128:(g + 1) * 128],
                            rhs=rhs_old,
                            start=False, stop=False,
                            skip_group_check=True,
                        )
                z_t = work.tile([128, C], F16, tag=f"z{s}", bufs=2,
                                name=f"z{s}")
                c_t = work.tile([128, C], F16, tag=f"c{s}", bufs=2,
                                name=f"c{s}")
                for h in range(NHALF):
                    cs = slice(h * CH, (h + 1) * CH)
                    nc.vector.scalar_tensor_tensor(
                        out=z_t[:, cs], in0=ifo_t[:, 3, cs], scalar=0.5,
                        in1=ifo_t[:, 0, cs], op0=sub, op1=mult)
                    for u in range(h * BH, (h + 1) * BH):
                        nc.vector.tensor_tensor_scan(
                            out=c_t[:, u * L:(u + 1) * L],
                            data0=ifo_t[:, 1, u * L:(u + 1) * L],
                            data1=z_t[:, u * L:(u + 1) * L],
                            initial=carry_c[s][:, u:u + 1],
                            op0=mult, op1=add,
                        )
                return ifo_t, z_t, c_t

            def emit_sweep_post(s, blk, sw, pre):
                t0 = blk * L
                last = sw == k_sweeps - 1
                ifo_t, z_t, c_t = pre
                o_v = ifo_t[:, 2, :].rearrange("p (u t) -> p u t", u=BS)
                if sw == 0 and k_sweeps > 2:
                    # Sweep-0 feedback h tolerates a crude tanh: its error
                    # contracts ~rho^2 (~0.07) before the output, so use
                    # 2*clamp(c/2, +-0.5) on the DVE and skip the ScalarE
                    # tanh entirely (ScalarE is the bottleneck engine).
                    cl_t = work.tile([128, C], F16, tag=f"cl{s}", bufs=2,
                                     name=f"cl{s}")
                    nc.vector.tensor_scalar(
                        cl_t, c_t, 0.5, -0.5,
                        op0=mybir.AluOpType.min, op1=mybir.AluOpType.max)
                    cl_v = cl_t.rearrange("p (u t) -> p u t", u=BS)
                    hsN = hs[s][0]
                    nc.vector.scalar_tensor_tensor(
                        out=hsN[:, :, 1:L + 1], in0=cl_v, scalar=2.0,
                        in1=o_v, op0=mult, op1=mult)
                    rhs = hs[s][0][:, :, 0:L]
                    for g in range(4):
                        nc.tensor.matmul(
                            ps[s][:, g, :],
                            lhsT=whh_sb[:, g * 128:(g + 1) * 128],
                            rhs=rhs,
                            start=False, stop=False,
                            skip_group_check=True,
                        )
                    return
                tc_t = work.tile([128, C], F16, tag=f"tc{s}", bufs=2,
                                 name=f"tc{s}")
                for h in range(NHALF):
                    cs = slice(h * CH, (h + 1) * CH)
                    nc.scalar.activation(out=tc_t[:, cs], in_=c_t[:, cs],
                                         func=tanh, scale=2.0)
                tc_v = tc_t.rearrange("p (u t) -> p u t", u=BS)
                if last:
                    out_t = work.tile([128, BS, L], out_dt, tag=f"out{s}",
                                      bufs=2, name=f"out{s}")
                    eng_out = nc.gpsimd if GP_OUT else nc.vector
                    for h in range(NHALF):
                        us = slice(h * BH, (h + 1) * BH)
                        eng_out.tensor_mul(out_t[:, us], o_v[:, us],
                                           tc_v[:, us])
                    u0 = s * BS
                    if not NO_OUT_DMA:
                        nc.sync.dma_start(
                            out=out_r[:, u0:u0 + BS, t0:t0 + L], in_=out_t)
                    if blk < NBLK - 1:
                        nc.gpsimd.tensor_copy(out=hs[s][0][:, :, 0],
                                              in_=out_t[:, :, L - 1])
                        if PE_DELTA:
                            nc.gpsimd.tensor_copy(out=hs[s][1][:, :, 0],
                                                  in_=out_t[:, :, L - 1])
                        nc.gpsimd.tensor_copy(
                            out=carry_c[s],
                            in_=c_t.rearrange(
                                "p (u t) -> p u t", u=BS)[:, :, L - 1])
                    return
                stop_all = sw == k_sweeps - 2
                eng_d = nc.gpsimd if GP_DELTA else nc.vector
                for h in range(NHALF):
                    us = slice(h * BH, (h + 1) * BH)
                    hsN = hs[s][sw % 2]
                    nc.vector.tensor_mul(hsN[:, us, 1:L + 1], o_v[:, us],
                                         tc_v[:, us])
                    if sw == 0:
                        rhs = hs[s][0][:, us, 0:L]
                    elif PE_DELTA:
                        rhs = hs[s][sw % 2][:, us, 0:L]
                    else:
                        eng_d.tensor_sub(
                            dlt[s][sw % 2][:, us, 1:L],
                            hs[s][sw % 2][:, us, 1:L],
                            hs[s][(sw + 1) % 2][:, us, 1:L])
                        rhs = dlt[s][sw % 2][:, us, 0:L]
                    for g in range(4):
                        nc.tensor.matmul(
                            ps[s][:, g, h * CH:(h + 1) * CH],
                            lhsT=whh_sb[:, g * 128:(g + 1) * 128],
                            rhs=rhs,
                            start=False,
                            stop=(stop_all and g == 3 and h == NHALF - 1),
                            skip_group_check=True,
                        )

            total_items = NBLK * (k_sweeps + 1)
            for _rep in range(reps):
              for s in range(S):
                nc.vector.memset(carry_c[s], 0.0)
                nc.gpsimd.memset(hs[s][0][:, :, 0], 0.0)
                if PE_DELTA:
                    nc.gpsimd.memset(hs[s][1][:, :, 0], 0.0)
              for t in range(total_items + (S - 1) * STAGGER):
                  pres = {}
                  for s in range(S):
                      idx = t - s * STAGGER
                      if not (0 <= idx < total_items):
                          continue
                      blk, ph = divmod(idx, k_sweeps + 1)
                      if ph == 0:
                          emit_xg(s, blk)
                      else:
                          pres[s] = (blk, ph - 1,
                                     emit_sweep_pre(s, blk, ph - 1))
                  for s, (blk, sw, pre) in pres.items():
                      emit_sweep_post(s, blk, sw, pre)

    if not nc.is_finalized():
        nc.finalize()
    return nc


def _get_nc(reps=1):
    key = f"nc{reps}"
    if key not in _NC_CACHE:
        _NC_CACHE[key] = _build_nc(reps=reps)
    return _NC_CACHE[key]


def _flip_padded(x, lengths):
    t = np.arange(x.shape[1])[None, :]
    Ln = lengths[:, None].astype(np.int64)
    idx = np.where(t < Ln, Ln - 1 - t, t)
    return np.take_along_axis(x, idx[:, :, None], axis=1)


def _pack_weights(W_ih, W_hh, b_ih, b_hh):
    # chunk order (i, f, o, g); the g chunk is pre-scaled by 2 because the
    # kernel computes tanh(g) as 2*sigmoid(2g) - 1 inside the fused sigmoid
    # instruction.
    Wi = W_ih.reshape(4, H, I)[PERM].copy()             # [4,128,256]
    Wi[3] *= 2.0
    wih = np.ascontiguousarray(
        Wi.transpose(2, 0, 1).reshape(2, 128, G4)).astype(np.float16)
    Wh = W_hh.reshape(4, H, H)[PERM].copy()             # [4,128,128]
    Wh[3] *= 2.0
    whh = np.ascontiguousarray(
        Wh.transpose(2, 0, 1).reshape(128, G4)).astype(np.float16)
    b4 = (b_ih + b_hh).reshape(4, H)[PERM].copy()
    b4[3] *= 2.0
    b = b4.reshape(1, G4).astype(np.float16)
    return wih, whh, np.ascontiguousarray(b)


def _pack_x(x_shard):
    # [U, T, I] -> [2, 128, U*T] with cols (u, t) u-major
    a = x_shard.transpose(2, 0, 1).reshape(2, 128, U * T)
    return np.ascontiguousarray(a).astype(np.float16)


def _run(inputs, trace=False):
    x = np.asarray(inputs["x"], np.float32)
    lengths = np.asarray(inputs["lengths"])
    Wf_ih = np.asarray(inputs["Wf_ih"], np.float32)
    Wf_hh = np.asarray(inputs["Wf_hh"], np.float32)
    bf_ih = np.asarray(inputs["bf_ih"], np.float32)
    bf_hh = np.asarray(inputs["bf_hh"], np.float32)
    Wb_ih = np.asarray(inputs["Wb_ih"], np.float32)
    Wb_hh = np.asarray(inputs["Wb_hh"], np.float32)
    bb_ih = np.asarray(inputs["bb_ih"], np.float32)
    bb_hh = np.asarray(inputs["bb_hh"], np.float32)

    x_rev = _flip_padded(x, lengths)
    wf = _pack_weights(Wf_ih, Wf_hh, bf_ih, bf_hh)
    wb = _pack_weights(Wb_ih, Wb_hh, bb_ih, bb_hh)

    in_maps = []
    for c in range(NCORES):
        if c < 4:
            xs = x[c * U:(c + 1) * U]
            wih, whh, b = wf
        else:
            xs = x_rev[(c - 4) * U:(c - 3) * U]
            wih, whh, b = wb
        in_maps.append({
            "xt": _pack_x(xs),
            "wih": wih,
            "whh": whh,
            "bias": b,
        })

    nc = _get_nc()
    res = run_bass_kernel_spmd(nc, in_maps, core_ids=list(range(NCORES)),
                               trace=trace)
    halves = []
    for c in range(NCORES):
        o = res.results[c]["out"].reshape(128, U, T).transpose(1, 2, 0)
        halves.append(o.astype(np.float32))
    fwd = np.concatenate(halves[0:4], axis=0)   # [32, T, 128]
    bwd = np.concatenate(halves[4:8], axis=0)   # [32, T, 128]
    out = np.concatenate([fwd, bwd], axis=-1).astype(np.float32)
    return out, res.exec_time_ns


def kernel(**inputs):
    out, _ = _run(inputs, trace=False)
    return out
